# revision 6
# baseline (speedup 1.0000x reference)
"""Gaussian resampling kernel for Trainium2 (8 NeuronCores, SPMD).

Computes, for each batch row b:
    e = cumsum(d); c = e - d/2
    w[t, s] = softmax_s(-(t - c_s)^2 / 10)   (masked s get weight 0)
    out[t, :] = sum_s w[t, s] * x[s, :]

Strategy:
  - Host precomputes c (float64 cumsum) and folds the mask in by moving
    masked centers to -1e4 (their exp underflows to exactly 0 in fp32).
  - Data-parallel over batch: 2 batches per core on 8 cores.
  - On device, scores are built in [S, T] layout (tokens on partitions):
    two ACT passes (Square with per-partition bias, then Exp).
  - A ones-column appended to x makes the matmul produce both the
    numerator (T, D) and the softmax denominator (T, 1) in one PSUM tile;
    normalization is a reciprocal + per-partition scalar multiply.
  - Matmuls run in float32r (fp32 data, full-rate PE mode).
"""

import math
import sys
import types

import numpy as np

# ---------------------------------------------------------------------------
# Optional NTFF-profiling plumbing. The runtime image lacks
# antenv.axon_hooks; wire a stand-in so run_bass_kernel_spmd(trace=True)
# works (used by the dev harness; the plain kernel path never traces).
try:  # pragma: no cover - best effort
    import antenv.axon_hooks  # noqa: F401
except ImportError:
    try:
        _hooks_mod = types.ModuleType("antenv.axon_hooks")
        _hook_box = [None]
        _hooks_mod.set_axon_ntff_profile_hook = (
            lambda hook: _hook_box.__setitem__(0, hook)
        )
        _hooks_mod.get_axon_ntff_profile_hook = lambda: _hook_box[0]
        sys.modules["antenv.axon_hooks"] = _hooks_mod
        from trn_agent_boot.trn_boot import _ntff_profile_via_ctypes

        _hooks_mod.set_axon_ntff_profile_hook(
            _ntff_profile_via_ctypes("/opt/axon/libaxon_pjrt.so")
        )
    except Exception:
        pass

import concourse.bacc as bacc
import concourse.mybir as mybir
import concourse.tile as tile
import concourse.bass_utils as bass_utils

# Avoid S3 artifact uploads from the trace path in this container.
bass_utils.upload_artifacts = lambda tmpdir: f"local:{tmpdir}"

from concourse.bass_utils import run_bass_kernel_spmd

NCORES = 8
B, S, D, T = 16, 512, 768, 4096
VARIANCE = 10.0
BPC = B // NCORES          # batches per core
P = 128                    # partitions
KC = S // P                # token chunks (4)
MC = T // P                # output frame chunks (32)
DW = D + 1                 # x with ones column appended
N0 = 512                   # first matmul column split (one PSUM bank)
TQ = T // 4                # ACT pass granularity along T

_PROGRAM = None


def _build_program():
    nc = bacc.Bacc("TRN2", target_bir_lowering=False, debug=False)
    f32 = mybir.dt.float32
    bf16 = mybir.dt.bfloat16

    xw_d = nc.dram_tensor("xw", [BPC, S, DW], bf16, kind="ExternalInput").ap()
    bias_d = nc.dram_tensor("bias", [BPC, S], f32, kind="ExternalInput").ap()
    trow_d = nc.dram_tensor("trow", [P, T], f32, kind="ExternalInput").ap()
    out_d = nc.dram_tensor("out", [BPC, T, D], f32, kind="ExternalOutput").ap()

    rsv = 1.0 / math.sqrt(VARIANCE)
    AF = mybir.ActivationFunctionType

    with tile.TileContext(nc) as tc:
        with tc.tile_pool(name="const", bufs=1) as constp, \
             tc.tile_pool(name="sb", bufs=2) as sb, \
             tc.tile_pool(name="outp", bufs=4) as outp, \
             tc.tile_pool(name="colp", bufs=4) as colp, \
             tc.tile_pool(name="ps", bufs=4, space="PSUM") as ps:

            trow = constp.tile([P, T], f32)
            for q in range(T // TQ):
                nc.sync.dma_start(
                    out=trow[:, q * TQ:(q + 1) * TQ],
                    in_=trow_d[:, q * TQ:(q + 1) * TQ],
                )

            for b in range(BPC):
                xw = sb.tile([P, KC, DW], bf16, name="xw_t", tag="xw_t")
                nc.sync.dma_start(
                    out=xw[:], in_=xw_d[b].rearrange("(k p) d -> p k d", p=P)
                )
                bcol = colp.tile([P, KC], f32, name="bcol", tag="bcol")
                nc.sync.dma_start(
                    out=bcol[:], in_=bias_d[b].rearrange("(k p) -> p k", p=P)
                )

                scores_r = sb.tile([P, KC, T], bf16, name="scores", tag="scores")
                for q in range(T // TQ):
                    for k in range(KC):
                        u = sb.tile([P, TQ], f32, name="u", tag="u", bufs=3)
                        slr = scores_r[:, k, q * TQ:(q + 1) * TQ]
                        nc.scalar.activation(
                            u[:], trow[:, q * TQ:(q + 1) * TQ], AF.Square,
                            bias=bcol[:, k:k + 1], scale=rsv,
                        )
                        nc.scalar.activation(slr, u[:], AF.Exp, scale=-1.0)
                for m in range(MC):
                    pt = ps.tile([P, DW], f32, name="pt", tag="pt")
                    for k in range(KC):
                        lhsT = scores_r[:, k, m * P:(m + 1) * P]
                        nc.tensor.matmul(
                            pt[:, 0:N0], lhsT, xw[:, k, 0:N0],
                            start=(k == 0), stop=(k == KC - 1),
                        )
                        nc.tensor.matmul(
                            pt[:, N0:DW], lhsT, xw[:, k, N0:DW],
                            start=(k == 0), stop=(k == KC - 1),
                        )
                    rcol = colp.tile([P, 1], f32, name="rcol", tag="rcol")
                    nc.vector.reciprocal(rcol[:], pt[:, D:DW])
                    ot = outp.tile([P, D], f32, name="ot", tag="ot")
                    nc.vector.tensor_scalar_mul(ot[:], pt[:, 0:D], rcol[:])
                    nc.sync.dma_start(
                        out=out_d[b, m * P:(m + 1) * P, :], in_=ot[:]
                    )

    nc.compile()
    return nc


def _get_program():
    global _PROGRAM
    if _PROGRAM is None:
        _PROGRAM = _build_program()
    return _PROGRAM


def _prepare_in_maps(x, d, mask):
    x = np.asarray(x, dtype=np.float32)
    d64 = np.asarray(d, dtype=np.float64)
    mask = np.asarray(mask, dtype=bool)

    e = np.cumsum(d64, axis=-1)
    c = e - 0.5 * d64                      # (B, S) token centers
    c = np.where(mask, c, -1.0e4)          # masked tokens: exp underflows to 0
    bias = (-c / math.sqrt(VARIANCE)).astype(np.float32)

    import ml_dtypes
    xw = np.empty((B, S, DW), dtype=ml_dtypes.bfloat16)
    xw[:, :, :D] = x.astype(ml_dtypes.bfloat16)
    xw[:, :, D] = 1.0

    trow = np.broadcast_to(
        np.arange(1, T + 1, dtype=np.float32)[None, :], (P, T)
    ).copy()

    in_maps = []
    for core in range(NCORES):
        sl = slice(core * BPC, (core + 1) * BPC)
        in_maps.append({
            "xw": xw[sl],
            "bias": bias[sl],
            "trow": trow,
        })
    return in_maps


def run(x, d, mask, frame_length, trace=False):
    assert int(frame_length) == T
    nc = _get_program()
    in_maps = _prepare_in_maps(x, d, mask)
    res = run_bass_kernel_spmd(nc, in_maps, list(range(NCORES)), trace=trace)
    out = np.empty((B, T, D), dtype=np.float32)
    for core in range(NCORES):
        out[core * BPC:(core + 1) * BPC] = res.results[core]["out"]
    return out, res


def kernel(x, d, mask, frame_length):
    out, _ = run(x, d, mask, frame_length, trace=False)
    return out


# revision 7
# speedup vs baseline: 1.1001x; 1.1001x over previous
"""Gaussian resampling kernel for Trainium2 (8 NeuronCores, SPMD).

Computes, for each batch row b:
    e = cumsum(d); c = e - d/2
    w[t, s] = softmax_s(-(t - c_s)^2 / 10)   (masked s get weight 0)
    out[t, :] = sum_s w[t, s] * x[s, :]

Strategy:
  - Host precomputes c (float64 cumsum) and folds the mask in by moving
    masked centers to -1e4 (their exp underflows to exactly 0 in fp32).
  - Data-parallel over batch: 2 batches per core on 8 cores.
  - Scores are built in [S, T] layout (tokens on partitions): two ACT
    passes (Square with per-partition bias, then Exp emitting bf16).
  - Banded sparsity: centers are monotone, so each 128-token chunk only
    has non-underflowing scores in a contiguous frame range. The bands
    (unioned over all batches, so the SPMD program is shared) are
    computed on the host from the actual durations and baked into the
    program; score/matmul work outside the bands is skipped. Skipped
    terms are exactly 0 in fp32, so this matches the dense reference.
  - A ones-column appended to x makes the matmul produce the numerator
    (T, D) and softmax denominator (T, 1) in one PSUM tile.
    Normalization = reciprocal (DVE) + per-partition scalar multiply
    (split 2:1 between DVE and ACT to balance engine load).
  - Matmuls in bf16; each (m, k) stationary is loaded once and reused
    for both N-pieces via ldweights=False on the second piece.
"""

import math
import sys
import types

import numpy as np

# ---------------------------------------------------------------------------
# Optional NTFF-profiling plumbing. The runtime image lacks
# antenv.axon_hooks; wire a stand-in so run_bass_kernel_spmd(trace=True)
# works (used by the dev harness; the plain kernel path never traces).
try:  # pragma: no cover - best effort
    import antenv.axon_hooks  # noqa: F401
except ImportError:
    try:
        _hooks_mod = types.ModuleType("antenv.axon_hooks")
        _hook_box = [None]
        _hooks_mod.set_axon_ntff_profile_hook = (
            lambda hook: _hook_box.__setitem__(0, hook)
        )
        _hooks_mod.get_axon_ntff_profile_hook = lambda: _hook_box[0]
        sys.modules["antenv.axon_hooks"] = _hooks_mod
        from trn_agent_boot.trn_boot import _ntff_profile_via_ctypes

        _hooks_mod.set_axon_ntff_profile_hook(
            _ntff_profile_via_ctypes("/opt/axon/libaxon_pjrt.so")
        )
    except Exception:
        pass

import concourse.bacc as bacc
import concourse.mybir as mybir
import concourse.tile as tile
import concourse.bass_utils as bass_utils
from concourse.tile_rust import add_dep_helper

# Avoid S3 artifact uploads from the trace path in this container.
bass_utils.upload_artifacts = lambda tmpdir: f"local:{tmpdir}"

from concourse.bass_utils import run_bass_kernel_spmd

NCORES = 8
B, S, D, T = 16, 512, 768, 4096
VARIANCE = 10.0
BPC = B // NCORES          # batches per core
P = 128                    # partitions
KC = S // P                # token chunks (4)
MC = T // P                # output frame chunks (32)
DW = D + 1                 # x with ones column appended
N0 = 512                   # first matmul column split (one PSUM bank)
MARGIN = 40.0              # frames; exp(-40^2/10) underflows fp32 to 0
ACT_PIECE = 2048           # max free-dim length of one score ACT op
OG = 4                     # m-chunks grouped per output DMA

_PROGRAMS = {}


def _compute_bands(c_masked):
    """Per token-chunk [lo, hi) active frame range (128-aligned), unioned
    over all batches. c_masked: (B, S) float64 with masked tokens = nan."""
    bands = []
    for k in range(KC):
        ck = c_masked[:, k * P:(k + 1) * P]
        lo = np.nanmin(ck) - MARGIN
        hi = np.nanmax(ck) + MARGIN
        a = max(0, int(math.floor(lo - 1)) // P * P)
        b = min(T, -(-int(math.ceil(hi)) // P) * P)
        b = max(b, a + P)
        bands.append((a, b))
    return tuple(bands)


def _build_program(bands):
    nc = bacc.Bacc("TRN2", target_bir_lowering=False, debug=False)
    f32 = mybir.dt.float32
    bf16 = mybir.dt.bfloat16

    xw_d = nc.dram_tensor("xw", [BPC, S, DW], bf16, kind="ExternalInput").ap()
    bias_d = nc.dram_tensor("bias", [BPC, S], f32, kind="ExternalInput").ap()
    out_d = nc.dram_tensor("out", [BPC, T, D], f32, kind="ExternalOutput").ap()

    rsv = 1.0 / math.sqrt(VARIANCE)
    AF = mybir.ActivationFunctionType

    # score pieces (k, t0, t1) in frame order; matmul chunk lists per m
    pieces = []
    for k, (a, b) in enumerate(bands):
        t0 = a
        while t0 < b:
            t1 = min(t0 + ACT_PIECE, b)
            pieces.append((k, t0, t1))
            t0 = t1
    pieces.sort(key=lambda p: (p[1], p[0]))
    mk = []
    for m in range(MC):
        ks = [k for k, (a, b) in enumerate(bands)
              if m * P < b and (m + 1) * P > a]
        assert ks, f"no active token chunk for m={m}"
        mk.append(ks)

    with tile.TileContext(nc) as tc:
        with tc.tile_pool(name="const", bufs=1) as constp, \
             tc.tile_pool(name="sb", bufs=2) as sb, \
             tc.tile_pool(name="outp", bufs=3) as outp, \
             tc.tile_pool(name="colp", bufs=4) as colp, \
             tc.tile_pool(name="ps", bufs=4, space="PSUM") as ps:

            trow = constp.tile([P, T], f32)
            nc.gpsimd.iota(trow[:], pattern=[[1, T]], base=1,
                           channel_multiplier=0,
                           allow_small_or_imprecise_dtypes=True)

            for b in range(BPC):
                xw = sb.tile([P, KC, DW], bf16, name="xw_t", tag="xw_t")
                nc.sync.dma_start(
                    out=xw[:], in_=xw_d[b].rearrange("(k p) d -> p k d", p=P)
                )
                bcol = colp.tile([P, KC], f32, name="bcol", tag="bcol")
                nc.sync.dma_start(
                    out=bcol[:], in_=bias_d[b].rearrange("(k p) -> p k", p=P)
                )

                scores = sb.tile([P, KC, T], bf16, name="scores", tag="scores")
                for k, t0, t1 in pieces:
                    u = sb.tile([P, t1 - t0], f32, name="u", tag="u", bufs=3)
                    nc.scalar.activation(
                        u[:], trow[:, t0:t1], AF.Square,
                        bias=bcol[:, k:k + 1], scale=rsv,
                    )
                    nc.scalar.activation(
                        scores[:, k, t0:t1], u[:], AF.Exp, scale=-1.0
                    )

                for mg in range(MC // OG):
                    ot = outp.tile([P, OG, D], f32, name="ot", tag="ot")
                    for g in range(OG):
                        m = mg * OG + g
                        ks = mk[m]
                        pt = ps.tile([P, DW], f32, name="pt", tag="pt")
                        for i, k in enumerate(ks):
                            lhsT = scores[:, k, m * P:(m + 1) * P]
                            st = (i == 0)
                            sp = (i == len(ks) - 1)
                            mma = nc.tensor.matmul(
                                pt[:, 0:N0], lhsT, xw[:, k, 0:N0],
                                start=st, stop=sp,
                            )
                            mmb = nc.tensor.matmul(
                                pt[:, N0:DW], lhsT, xw[:, k, N0:DW],
                                start=st, stop=sp,
                            )
                            mmb.ins.ldweights = False
                            add_dep_helper(mmb.ins, mma.ins,
                                           reason="reuse loaded weights")
                        rcol = colp.tile([P, 1], f32, name="rcol", tag="rcol")
                        nc.vector.reciprocal(rcol[:], pt[:, D:DW])
                        if m % 3 == 2:
                            nc.scalar.activation(
                                ot[:, g, :], pt[:, 0:D], AF.Copy,
                                scale=rcol[:],
                            )
                        else:
                            nc.vector.tensor_scalar_mul(
                                ot[:, g, :], pt[:, 0:D], rcol[:]
                            )
                    nc.sync.dma_start(
                        out=out_d[b, mg * OG * P:(mg + 1) * OG * P, :]
                        .rearrange("(g p) d -> p g d", p=P),
                        in_=ot[:],
                    )

    nc.compile()
    return nc


def _get_program(bands):
    prog = _PROGRAMS.get(bands)
    if prog is None:
        prog = _build_program(bands)
        _PROGRAMS[bands] = prog
    return prog


def _prepare(x, d, mask):
    x = np.asarray(x, dtype=np.float32)
    d64 = np.asarray(d, dtype=np.float64)
    mask = np.asarray(mask, dtype=bool)

    e = np.cumsum(d64, axis=-1)
    c = e - 0.5 * d64                      # (B, S) token centers
    c_m = np.where(mask, c, np.nan)
    bands = _compute_bands(c_m)
    c = np.where(mask, c, -1.0e4)          # masked tokens: exp underflows to 0
    bias = (-c / math.sqrt(VARIANCE)).astype(np.float32)

    import ml_dtypes
    xw = np.empty((B, S, DW), dtype=ml_dtypes.bfloat16)
    xw[:, :, :D] = x.astype(ml_dtypes.bfloat16)
    xw[:, :, D] = 1.0

    in_maps = []
    for core in range(NCORES):
        sl = slice(core * BPC, (core + 1) * BPC)
        in_maps.append({"xw": xw[sl], "bias": bias[sl]})
    return in_maps, bands


def run(x, d, mask, frame_length, trace=False):
    assert int(frame_length) == T
    in_maps, bands = _prepare(x, d, mask)
    nc = _get_program(bands)
    res = run_bass_kernel_spmd(nc, in_maps, list(range(NCORES)), trace=trace)
    out = np.empty((B, T, D), dtype=np.float32)
    for core in range(NCORES):
        out[core * BPC:(core + 1) * BPC] = res.results[core]["out"]
    return out, res


def kernel(x, d, mask, frame_length):
    out, _ = run(x, d, mask, frame_length, trace=False)
    return out


# revision 11
# speedup vs baseline: 1.1523x; 1.0474x over previous
"""Gaussian resampling kernel for Trainium2 (8 NeuronCores, SPMD).

Computes, for each batch row b:
    e = cumsum(d); c = e - d/2
    w[t, s] = softmax_s(-(t - c_s)^2 / 10)   (masked s get weight 0)
    out[t, :] = sum_s w[t, s] * x[s, :]

Strategy:
  - Host precomputes c (float64 cumsum) and folds the mask in by moving
    masked centers to -1e4 (their exp underflows to exactly 0 in fp32).
  - Data-parallel over batch: 2 batches per core on 8 cores.
  - Scores are built in [S, T] layout (tokens on partitions): two ACT
    passes (Square with per-partition bias, then Exp emitting bf16).
  - Banded sparsity: centers are monotone, so each 128-token chunk only
    has non-underflowing scores in a contiguous frame range. The bands
    (unioned over all batches, so the SPMD program is shared) are
    computed on the host from the actual durations and baked into the
    program; score/matmul work outside the bands is skipped. Skipped
    terms are exactly 0 in fp32, so this matches the dense reference.
  - A ones-column appended to x makes the matmul produce the numerator
    (T, D) and softmax denominator (T, 1) in one PSUM tile.
    Normalization = reciprocal (DVE) + per-partition scalar multiply
    (split 2:1 between DVE and ACT to balance engine load).
  - Matmuls in bf16; each (m, k) stationary is loaded once and reused
    for both N-pieces via ldweights=False on the second piece.
"""

import math
import sys
import types

import numpy as np

# ---------------------------------------------------------------------------
# Optional NTFF-profiling plumbing. The runtime image lacks
# antenv.axon_hooks; wire a stand-in so run_bass_kernel_spmd(trace=True)
# works (used by the dev harness; the plain kernel path never traces).
try:  # pragma: no cover - best effort
    import antenv.axon_hooks  # noqa: F401
except ImportError:
    try:
        _hooks_mod = types.ModuleType("antenv.axon_hooks")
        _hook_box = [None]
        _hooks_mod.set_axon_ntff_profile_hook = (
            lambda hook: _hook_box.__setitem__(0, hook)
        )
        _hooks_mod.get_axon_ntff_profile_hook = lambda: _hook_box[0]
        sys.modules["antenv.axon_hooks"] = _hooks_mod
        from trn_agent_boot.trn_boot import _ntff_profile_via_ctypes

        _hooks_mod.set_axon_ntff_profile_hook(
            _ntff_profile_via_ctypes("/opt/axon/libaxon_pjrt.so")
        )
    except Exception:
        pass

import concourse.bacc as bacc
import concourse.mybir as mybir
import concourse.tile as tile
import concourse.bass_utils as bass_utils
from concourse.tile_rust import add_dep_helper

# Avoid S3 artifact uploads from the trace path in this container.
bass_utils.upload_artifacts = lambda tmpdir: f"local:{tmpdir}"

from concourse.bass_utils import run_bass_kernel_spmd

NCORES = 8
B, S, D, T = 16, 512, 768, 4096
VARIANCE = 10.0
BPC = B // NCORES          # batches per core
P = 128                    # partitions
KC = S // P                # token chunks (4)
MC = T // P                # output frame chunks (32)
DW = D + 1                 # x with ones column appended
N0 = 512                   # first matmul column split (one PSUM bank)
MARGIN = 40.0              # frames; exp(-40^2/10) underflows fp32 to 0
ACT_PIECE = 2048           # max free-dim length of one score ACT op
OG = 2                     # m-chunks grouped per output DMA

_PROGRAMS = {}


def _compute_bands(c_masked):
    """Per token-chunk [lo, hi) active frame range (128-aligned), unioned
    over all batches. c_masked: (B, S) float64 with masked tokens = nan."""
    bands = []
    for k in range(KC):
        ck = c_masked[:, k * P:(k + 1) * P]
        lo = np.nanmin(ck) - MARGIN
        hi = np.nanmax(ck) + MARGIN
        a = max(0, int(math.floor(lo - 1)) // P * P)
        b = min(T, -(-int(math.ceil(hi)) // P) * P)
        b = max(b, a + P)
        bands.append((a, b))
    return tuple(bands)


def _build_program(bands):
    nc = bacc.Bacc("TRN2", target_bir_lowering=False, debug=False)
    f32 = mybir.dt.float32
    bf16 = mybir.dt.bfloat16

    xw_d = nc.dram_tensor("xw", [BPC, S, DW], bf16, kind="ExternalInput").ap()
    bias_d = nc.dram_tensor("bias", [BPC, S], f32, kind="ExternalInput").ap()
    trow_d = nc.dram_tensor("trow", [P, T], f32, kind="ExternalInput").ap()
    out_d = nc.dram_tensor("out", [BPC, T, D], f32, kind="ExternalOutput").ap()

    rsv = 1.0 / math.sqrt(VARIANCE)
    AF = mybir.ActivationFunctionType

    # score pieces (k, t0, t1) in frame order; matmul chunk lists per m
    pieces = []
    for k, (a, b) in enumerate(bands):
        t0 = a
        while t0 < b:
            t1 = min(t0 + ACT_PIECE, b)
            pieces.append((k, t0, t1))
            t0 = t1
    pieces.sort(key=lambda p: (p[1], p[0]))
    mk = []
    for m in range(MC):
        ks = [k for k, (a, b) in enumerate(bands)
              if m * P < b and (m + 1) * P > a]
        assert ks, f"no active token chunk for m={m}"
        mk.append(ks)

    with tile.TileContext(nc) as tc:
        with tc.tile_pool(name="const", bufs=1) as constp, \
             tc.tile_pool(name="sb", bufs=2) as sb, \
             tc.tile_pool(name="outp", bufs=3) as outp, \
             tc.tile_pool(name="colp", bufs=4) as colp, \
             tc.tile_pool(name="ps", bufs=4, space="PSUM") as ps:

            trow = constp.tile([P, T], f32)
            nc.sync.dma_start(out=trow[:, 0:T // 2], in_=trow_d[:, 0:T // 2])

            for b in range(BPC):
                bcol = colp.tile([P, KC], f32, name="bcol", tag="bcol")
                nc.sync.dma_start(
                    out=bcol[:], in_=bias_d[b].rearrange("(k p) -> p k", p=P)
                )
                xw = sb.tile([P, KC, DW], bf16, name="xw_t", tag="xw_t")
                xw_src = xw_d[b].rearrange("(k p) d -> p k d", p=P)
                for k in range(KC):
                    nc.sync.dma_start(
                        out=xw[:, k:k + 1, :], in_=xw_src[:, k:k + 1, :]
                    )
                if b == 0:
                    nc.sync.dma_start(
                        out=trow[:, T // 2:T], in_=trow_d[:, T // 2:T]
                    )

                scores = sb.tile([P, KC, T], bf16, name="scores", tag="scores")
                for k, t0, t1 in pieces:
                    u = sb.tile([P, t1 - t0], f32, name="u", tag="u", bufs=3)
                    nc.scalar.activation(
                        u[:], trow[:, t0:t1], AF.Square,
                        bias=bcol[:, k:k + 1], scale=rsv,
                    )
                    nc.scalar.activation(
                        scores[:, k, t0:t1], u[:], AF.Exp, scale=-1.0
                    )

                for mg in range(MC // OG):
                    ot = outp.tile([P, OG, D], f32, name="ot", tag="ot")
                    for g in range(OG):
                        m = mg * OG + g
                        ks = mk[m]
                        pt = ps.tile([P, DW], f32, name="pt", tag="pt")
                        for i, k in enumerate(ks):
                            lhsT = scores[:, k, m * P:(m + 1) * P]
                            st = (i == 0)
                            sp = (i == len(ks) - 1)
                            mma = nc.tensor.matmul(
                                pt[:, 0:N0], lhsT, xw[:, k, 0:N0],
                                start=st, stop=sp,
                            )
                            mmb = nc.tensor.matmul(
                                pt[:, N0:DW], lhsT, xw[:, k, N0:DW],
                                start=st, stop=sp,
                            )
                            mmb.ins.ldweights = False
                            add_dep_helper(mmb.ins, mma.ins,
                                           reason="reuse loaded weights")
                        rcol = colp.tile([P, 1], f32, name="rcol", tag="rcol")
                        nc.vector.reciprocal(rcol[:], pt[:, D:DW])
                        if m % 3 == 2:
                            nc.scalar.activation(
                                ot[:, g, :], pt[:, 0:D], AF.Copy,
                                scale=rcol[:],
                            )
                        else:
                            nc.vector.tensor_scalar_mul(
                                ot[:, g, :], pt[:, 0:D], rcol[:]
                            )
                    nc.sync.dma_start(
                        out=out_d[b, mg * OG * P:(mg + 1) * OG * P, :]
                        .rearrange("(g p) d -> p g d", p=P),
                        in_=ot[:],
                    )

    nc.compile()
    return nc


def _get_program(bands):
    prog = _PROGRAMS.get(bands)
    if prog is None:
        prog = _build_program(bands)
        _PROGRAMS[bands] = prog
    return prog


def _prepare(x, d, mask):
    x = np.asarray(x, dtype=np.float32)
    d64 = np.asarray(d, dtype=np.float64)
    mask = np.asarray(mask, dtype=bool)

    e = np.cumsum(d64, axis=-1)
    c = e - 0.5 * d64                      # (B, S) token centers
    c_m = np.where(mask, c, np.nan)
    bands = _compute_bands(c_m)
    c = np.where(mask, c, -1.0e4)          # masked tokens: exp underflows to 0
    bias = (-c / math.sqrt(VARIANCE)).astype(np.float32)

    import ml_dtypes
    xw = np.empty((B, S, DW), dtype=ml_dtypes.bfloat16)
    xw[:, :, :D] = x.astype(ml_dtypes.bfloat16)
    xw[:, :, D] = 1.0

    trow = np.broadcast_to(
        np.arange(1, T + 1, dtype=np.float32)[None, :], (P, T)
    ).copy()

    in_maps = []
    for core in range(NCORES):
        sl = slice(core * BPC, (core + 1) * BPC)
        in_maps.append({"xw": xw[sl], "bias": bias[sl], "trow": trow})
    return in_maps, bands


def run(x, d, mask, frame_length, trace=False):
    assert int(frame_length) == T
    in_maps, bands = _prepare(x, d, mask)
    nc = _get_program(bands)
    res = run_bass_kernel_spmd(nc, in_maps, list(range(NCORES)), trace=trace)
    out = np.empty((B, T, D), dtype=np.float32)
    for core in range(NCORES):
        out[core * BPC:(core + 1) * BPC] = res.results[core]["out"]
    return out, res


def kernel(x, d, mask, frame_length):
    out, _ = run(x, d, mask, frame_length, trace=False)
    return out


# revision 14
# speedup vs baseline: 1.2158x; 1.0550x over previous
"""Gaussian resampling kernel for Trainium2 (8 NeuronCores, SPMD).

Computes, for each batch row b:
    e = cumsum(d); c = e - d/2
    w[t, s] = softmax_s(-(t - c_s)^2 / 10)   (masked s get weight 0)
    out[t, :] = sum_s w[t, s] * x[s, :]

Strategy:
  - Host precomputes c (float64 cumsum) and folds the mask in by moving
    masked centers to -1e4 (their exp underflows to exactly 0 in fp32).
  - Data-parallel over batch: 2 batches per core on 8 cores.
  - Scores are built in [S, T] layout (tokens on partitions): two ACT
    passes (Square with per-partition bias, then Exp emitting bf16).
  - Banded sparsity: centers are monotone, so each 128-token chunk only
    has non-underflowing scores in a contiguous frame range. The bands
    (unioned over all batches, so the SPMD program is shared) are
    computed on the host from the actual durations and baked into the
    program; score/matmul work outside the bands is skipped. Skipped
    terms are exactly 0 in fp32, so this matches the dense reference.
  - A ones-column appended to x makes the matmul produce the numerator
    (T, D) and softmax denominator (T, 1) in one PSUM tile.
    Normalization = reciprocal (DVE) + per-partition scalar multiply
    (split 2:1 between DVE and ACT to balance engine load).
  - Matmuls in bf16; each (m, k) stationary is loaded once and reused
    for both N-pieces via ldweights=False on the second piece.
"""

import math
import sys
import types

import numpy as np

# ---------------------------------------------------------------------------
# Optional NTFF-profiling plumbing. The runtime image lacks
# antenv.axon_hooks; wire a stand-in so run_bass_kernel_spmd(trace=True)
# works (used by the dev harness; the plain kernel path never traces).
try:  # pragma: no cover - best effort
    import antenv.axon_hooks  # noqa: F401
except ImportError:
    try:
        _hooks_mod = types.ModuleType("antenv.axon_hooks")
        _hook_box = [None]
        _hooks_mod.set_axon_ntff_profile_hook = (
            lambda hook: _hook_box.__setitem__(0, hook)
        )
        _hooks_mod.get_axon_ntff_profile_hook = lambda: _hook_box[0]
        sys.modules["antenv.axon_hooks"] = _hooks_mod
        from trn_agent_boot.trn_boot import _ntff_profile_via_ctypes

        _hooks_mod.set_axon_ntff_profile_hook(
            _ntff_profile_via_ctypes("/opt/axon/libaxon_pjrt.so")
        )
    except Exception:
        pass

import concourse.bacc as bacc
import concourse.mybir as mybir
import concourse.tile as tile
import concourse.bass_utils as bass_utils
from concourse.tile_rust import add_dep_helper

# Avoid S3 artifact uploads from the trace path in this container.
bass_utils.upload_artifacts = lambda tmpdir: f"local:{tmpdir}"

from concourse.bass_utils import run_bass_kernel_spmd

NCORES = 8
B, S, D, T = 16, 512, 768, 4096
VARIANCE = 10.0
BPC = B // NCORES          # batches per core
P = 128                    # partitions
KC = S // P                # token chunks (4)
MC = T // P                # output frame chunks (32)
DW = D + 1                 # x with ones column appended
N0 = 512                   # first matmul column split (one PSUM bank)
MARGIN = 40.0              # frames; exp(-40^2/10) underflows fp32 to 0
ACT_PIECE = 2048           # max free-dim length of one score ACT op
OG = 2                     # m-chunks grouped per output DMA

_PROGRAMS = {}


def _compute_bands(c_masked):
    """Per token-chunk [lo, hi) active frame range (128-aligned), unioned
    over the given batches. c_masked: (n, S) float64, masked tokens nan.
    A fully-masked chunk yields None (skipped entirely)."""
    bands = []
    for k in range(KC):
        ck = c_masked[:, k * P:(k + 1) * P]
        if np.all(np.isnan(ck)):
            bands.append(None)
            continue
        lo = np.nanmin(ck) - MARGIN
        hi = np.nanmax(ck) + MARGIN
        a = max(0, int(math.floor(lo - 1)) // P * P)
        b = min(T, -(-int(math.ceil(hi)) // P) * P)
        b = max(b, a + P)
        bands.append((a, b))
    return tuple(bands)


def _act_scale(b, m):
    """Which normalization multiplies run on ACT instead of DVE."""
    if b == 0:
        return m >= 6 and m % 3 == 2
    return m % 2 == 1


def _build_program(bands2):
    """bands2: per batch-slot tuple of per-chunk (a, b) bands (or None)."""
    nc = bacc.Bacc("TRN2", target_bir_lowering=False, debug=False)
    f32 = mybir.dt.float32
    bf16 = mybir.dt.bfloat16

    xw_d = nc.dram_tensor("xw", [BPC, S, DW], bf16, kind="ExternalInput").ap()
    bias_d = nc.dram_tensor("bias", [BPC, S], f32, kind="ExternalInput").ap()
    trow_d = nc.dram_tensor("trow", [P, T], f32, kind="ExternalInput").ap()
    out_d = nc.dram_tensor("out", [BPC, T, D], f32, kind="ExternalOutput").ap()

    rsv = 1.0 / math.sqrt(VARIANCE)
    AF = mybir.ActivationFunctionType

    # score pieces (k, t0, t1) in frame order; matmul chunk lists per m
    pieces2, mk2 = [], []
    for bands in bands2:
        pieces = []
        for k, band in enumerate(bands):
            if band is None:
                continue
            a, b = band
            t0 = a
            while t0 < b:
                t1 = min(t0 + ACT_PIECE, b)
                pieces.append((k, t0, t1))
                t0 = t1
        pieces.sort(key=lambda p: (p[1], p[0]))
        pieces2.append(pieces)
        mk = []
        for m in range(MC):
            ks = [k for k, band in enumerate(bands)
                  if band and m * P < band[1] and (m + 1) * P > band[0]]
            assert ks, f"no active token chunk for m={m}"
            mk.append(ks)
        mk2.append(mk)

    with tile.TileContext(nc) as tc:
        with tc.tile_pool(name="const", bufs=1) as constp, \
             tc.tile_pool(name="sb", bufs=2) as sb, \
             tc.tile_pool(name="outp", bufs=3) as outp, \
             tc.tile_pool(name="colp", bufs=4) as colp, \
             tc.tile_pool(name="ps", bufs=4, space="PSUM") as ps:

            # Warm the ACT table set (exp_and_others) before any real work.
            warm = colp.tile([P, 1], f32, name="warm", tag="warm", bufs=1)
            nc.vector.memset(warm[:], 0.0)
            nc.scalar.activation(warm[:], warm[:], AF.Exp)

            # All input DMAs up front. Batch 0's on the Sync queue (shared
            # with output DMAs), batch 1's on the GpSimd queue so they are
            # not stuck behind batch 0's in-order output issues.
            trow = constp.tile([P, T], f32)
            nc.sync.dma_start(out=trow[:, 0:T // 2], in_=trow_d[:, 0:T // 2])
            tiles = []
            for b in range(BPC):
                eng = nc.sync if b == 0 else nc.gpsimd
                bcol = colp.tile([P, KC], f32, name="bcol", tag="bcol")
                eng.dma_start(
                    out=bcol[:], in_=bias_d[b].rearrange("(k p) -> p k", p=P)
                )
                xw = sb.tile([P, KC, DW], bf16, name="xw_t", tag="xw_t")
                xw_src = xw_d[b].rearrange("(k p) d -> p k d", p=P)
                for k in range(KC):
                    eng.dma_start(
                        out=xw[:, k:k + 1, :], in_=xw_src[:, k:k + 1, :]
                    )
                if b == 0:
                    nc.sync.dma_start(
                        out=trow[:, T // 2:T], in_=trow_d[:, T // 2:T]
                    )
                tiles.append((bcol, xw))

            for b in range(BPC):
                bcol, xw = tiles[b]
                scores = sb.tile([P, KC, T], bf16, name="scores", tag="scores")
                for k, t0, t1 in pieces2[b]:
                    u = sb.tile([P, t1 - t0], f32, name="u", tag="u", bufs=3)
                    nc.scalar.activation(
                        u[:], trow[:, t0:t1], AF.Square,
                        bias=bcol[:, k:k + 1], scale=rsv,
                    )
                    nc.scalar.activation(
                        scores[:, k, t0:t1], u[:], AF.Exp, scale=-1.0
                    )

                for mg in range(MC // OG):
                    ot = outp.tile([P, OG, D], f32, name="ot", tag="ot")
                    for g in range(OG):
                        m = mg * OG + g
                        ks = mk2[b][m]
                        pt = ps.tile([P, DW], f32, name="pt", tag="pt")
                        for i, k in enumerate(ks):
                            lhsT = scores[:, k, m * P:(m + 1) * P]
                            st = (i == 0)
                            sp = (i == len(ks) - 1)
                            mma = nc.tensor.matmul(
                                pt[:, 0:N0], lhsT, xw[:, k, 0:N0],
                                start=st, stop=sp,
                            )
                            mmb = nc.tensor.matmul(
                                pt[:, N0:DW], lhsT, xw[:, k, N0:DW],
                                start=st, stop=sp,
                            )
                            add_dep_helper(mmb.ins, mma.ins,
                                           reason="keep N-pieces adjacent")
                        rcol = colp.tile([P, 1], f32, name="rcol", tag="rcol")
                        nc.vector.reciprocal(rcol[:], pt[:, D:DW])
                        if _act_scale(b, m):
                            nc.scalar.activation(
                                ot[:, g, :], pt[:, 0:D], AF.Copy,
                                scale=rcol[:],
                            )
                        else:
                            nc.vector.tensor_scalar_mul(
                                ot[:, g, :], pt[:, 0:D], rcol[:]
                            )
                    nc.sync.dma_start(
                        out=out_d[b, mg * OG * P:(mg + 1) * OG * P, :]
                        .rearrange("(g p) d -> p g d", p=P),
                        in_=ot[:],
                    )

    nc.compile()
    return nc


def _get_program(bands):
    prog = _PROGRAMS.get(bands)
    if prog is None:
        prog = _build_program(bands)
        _PROGRAMS[bands] = prog
    return prog


def _prepare(x, d, mask):
    x = np.asarray(x, dtype=np.float32)
    d64 = np.asarray(d, dtype=np.float64)
    mask = np.asarray(mask, dtype=bool)

    e = np.cumsum(d64, axis=-1)
    c = e - 0.5 * d64                      # (B, S) token centers
    c_m = np.where(mask, c, np.nan)

    # Sort batches by valid length; slot 0 takes the 8 shortest, slot 1 the
    # 8 longest. Similar lengths per slot give much tighter per-slot bands.
    order = np.argsort(mask.sum(1), kind="stable")
    bands2 = tuple(
        _compute_bands(c_m[order[s * NCORES:(s + 1) * NCORES]])
        for s in range(BPC)
    )

    c = np.where(mask, c, -1.0e4)          # masked tokens: exp underflows to 0
    bias = (-c / math.sqrt(VARIANCE)).astype(np.float32)

    import ml_dtypes
    xw = np.empty((B, S, DW), dtype=ml_dtypes.bfloat16)
    xw[:, :, :D] = x.astype(ml_dtypes.bfloat16)
    xw[:, :, D] = 1.0

    trow = np.broadcast_to(
        np.arange(1, T + 1, dtype=np.float32)[None, :], (P, T)
    ).copy()

    in_maps = []
    for core in range(NCORES):
        idx = [order[core], order[NCORES + core]]
        in_maps.append({
            "xw": np.ascontiguousarray(xw[idx]),
            "bias": np.ascontiguousarray(bias[idx]),
            "trow": trow,
        })
    return in_maps, bands2, order


def run(x, d, mask, frame_length, trace=False):
    assert int(frame_length) == T
    in_maps, bands2, order = _prepare(x, d, mask)
    nc = _get_program(bands2)
    res = run_bass_kernel_spmd(nc, in_maps, list(range(NCORES)), trace=trace)
    out = np.empty((B, T, D), dtype=np.float32)
    for core in range(NCORES):
        for s in range(BPC):
            out[order[s * NCORES + core]] = res.results[core]["out"][s]
    return out, res


def kernel(x, d, mask, frame_length):
    out, _ = run(x, d, mask, frame_length, trace=False)
    return out
